# revision 1
# baseline (speedup 1.0000x reference)
"""MiniGPT forward on 8 Trainium2 NeuronCores.

Sharding: core c handles sequence (c & 3) and vocab half (c >> 2).  The 6
transformer blocks are data-parallel over the 4 sequences (each pair of
cores duplicates block compute); the tied-embedding LM head is split over
the vocab.  No collectives.

Blocks run in bf16 (weights + activations, f32 residual stream / PSUM):
LayerNorm gains are folded into the following linear's weights on the
host; LN stats use PE ones-matmuls with the x^2 squares on gpsimd; causal
masking is applied on the PE by accumulating I @ (-240*(p>f)) into the
score PSUM (no vector/gpsimd masking).  The LM head runs as fp8(e4m3)
DoubleRow matmuls with error feedback: logits = Wh@xh + Wh@xl + Wl@xh
where xh/xl and Wh/Wl are hi/lo e4m3 splits — W-16-bit-equivalent
accuracy at 0.75x the bf16 PE cost, fully hidden under the 105MB logits
output DMA.  Logits stream out via one batched DMA per 512-vocab chunk.
"""

import sys

sys.path.insert(0, "/opt/trn_rl_repo")

import numpy as np
import ml_dtypes

import concourse.bacc as bacc
import concourse.tile as tile
from concourse import mybir
from concourse.bass_utils import run_bass_kernel_spmd

F32 = mybir.dt.float32
F32R = mybir.dt.float32r
BF16 = mybir.dt.bfloat16
FP8 = mybir.dt.float8e4
ALU = mybir.AluOpType
ACT = mybir.ActivationFunctionType
DR = mybir.MatmulPerfMode.DoubleRow
E4 = ml_dtypes.float8_e4m3

B, T, C, H, HD, L, V = 4, 1024, 768, 12, 64, 6, 50257
CT = C // 128           # 6 c-tiles
KP = CT // 2            # 3 k-pairs (fp8 head)
TT = T // 128           # 8 token tiles
AQ = 256                # attention query block
HT = 3072 // 128        # 24 hidden tiles
VS = 25600              # vocab shard per core
NVC = VS // 512         # 50
EPS = 1e-5
SXH = 16.0              # fp8 scale for the head's x

_CACHE = {}
LAST_RESULT = None
LAST_NC = None


def build_program(sc, n_layers=L):
    nc = bacc.Bacc(None, target_bir_lowering=False)

    def bf_in(name, shape):
        return nc.dram_tensor(name, list(shape), BF16, kind="ExternalInput")

    x0t_d = nc.dram_tensor("x0t", [128, CT, T], F32R, kind="ExternalInput")
    wq, wv_, wp, wf, wf2 = [], [], [], [], []
    for l in range(n_layers):
        wq.append(bf_in(f"qkw{l}", (12 * 128, CT, 128)))
        wv_.append(bf_in(f"vw{l}", (128, CT, C)))
        wp.append(bf_in(f"pw{l}", (CT * 128, CT, 128)))
        wf.append(bf_in(f"fw{l}", (HT * 128, CT, 128)))
        wf2.append(bf_in(f"f2w{l}", (CT * 128, HT, 128)))
    whh_d = nc.dram_tensor("whh", [NVC * 128, KP, 2, 512], FP8,
                           kind="ExternalInput")
    whl_d = nc.dram_tensor("whl", [NVC * 128, KP, 2, 512], FP8,
                           kind="ExternalInput")
    masks_d = nc.dram_tensor("masks", [128, 2, AQ], FP8,
                             kind="ExternalInput")
    iden_d = nc.dram_tensor("iden", [128, 2, 128], FP8,
                            kind="ExternalInput")
    logits_d = nc.dram_tensor("logits", [T, VS], F32, kind="ExternalOutput")

    ln_ctr = [0]

    with nc.allow_low_precision("bf16 blocks + fp8 head intentional"), \
         tile.TileContext(nc) as tc:
        glob = tc.alloc_tile_pool(name="glob", bufs=1)
        gx = tc.alloc_tile_pool(name="gx", bufs=1)
        gx2 = tc.alloc_tile_pool(name="gx2", bufs=3)
        gmicro = tc.alloc_tile_pool(name="gmicro", bufs=1)
        ps_big = tc.alloc_tile_pool(name="ps_big", bufs=3, space="PSUM")
        gxn = tc.alloc_tile_pool(name="gxn", bufs=1)
        gw = tc.alloc_tile_pool(name="gw", bufs=5)
        gwv = tc.alloc_tile_pool(name="gwv", bufs=2)
        gw24 = tc.alloc_tile_pool(name="gw24", bufs=3)

        ones_col = glob.tile([128, 1], F32R, tag="ones_col")
        ones_row = glob.tile([1, 128], F32R, tag="ones_row")
        eps_t = glob.tile([1, 1], F32, tag="eps")
        epsh_t = glob.tile([1, 1], F32, tag="epsh")
        masks_t = glob.tile([128, 2, AQ], FP8, tag="masks")
        iden_t = glob.tile([128, 2, 128], FP8, tag="iden")
        nc.vector.memset(ones_col[:].bitcast(F32), 1.0)
        nc.vector.memset(ones_row[:].bitcast(F32), 1.0)
        nc.vector.memset(eps_t[:], EPS)
        nc.vector.memset(epsh_t[:], EPS / (SXH * SXH))
        nc.sync.dma_start(out=masks_t[:], in_=masks_d[:])
        nc.sync.dma_start(out=iden_t[:], in_=iden_d[:])

        xT = gx.tile([128, CT, T], F32R, tag="xT")
        nc.sync.dma_start(out=xT[:], in_=x0t_d[:])

        # persistent v tile: [p, tt, h, 0:64] = v ; [.., 64:128] = 1.0
        gv = tc.alloc_tile_pool(name="gv", bufs=1)
        vT = gv.tile([128, TT, H, 128], BF16, tag="vT")
        nc.gpsimd.memset(vT[:, :, :, 64:128], 1.0)

        def ln_qc(xin, xout, eps_ap, sx, qc, ps_stat, ps_bc):
            """one token-half of xout = (xin - mu) * rstd * sx."""
            if True:
                qs = slice(qc * 512, (qc + 1) * 512)
                s_ps = ps_stat.tile([1, 512], F32, space="PSUM", tag="stat")
                q_ps = ps_stat.tile([1, 512], F32, space="PSUM", tag="stat")
                for kt in range(CT):
                    nc.tensor.matmul(s_ps[:], ones_col[:], xin[:, kt, qs],
                                     start=(kt == 0), stop=(kt == CT - 1))
                for kt in range(CT):
                    x2 = gx2.tile([128, 512], F32R, tag="x2")
                    nc.gpsimd.tensor_tensor(
                        out=x2[:], in0=xin[:, kt, qs],
                        in1=xin[:, kt, qs], op=ALU.mult)
                    nc.tensor.matmul(q_ps[:], ones_col[:], x2[:],
                                     start=(kt == 0), stop=(kt == CT - 1))
                mu = gmicro.tile([1, 512], F32R, tag="mu")
                nc.scalar.mul(mu[:], s_ps[:], 1.0 / C)
                mu2 = gmicro.tile([1, 512], F32, tag="mu2")
                nc.scalar.activation(mu2[:], mu[:], ACT.Square)
                var = gmicro.tile([1, 512], F32, tag="var")
                nc.vector.scalar_tensor_tensor(
                    out=var[:], in0=q_ps[:], scalar=1.0 / C, in1=mu2[:],
                    op0=ALU.mult, op1=ALU.subtract)
                sd = gmicro.tile([1, 512], F32, tag="sd")
                nc.scalar.activation(sd[:], var[:], ACT.Sqrt, bias=eps_ap,
                                     scale=1.0 / (sx * sx))
                r = gmicro.tile([1, 512], F32R, tag="r")
                nc.vector.reciprocal(r[:], sd[:])            # sx/sd
                mr = gmicro.tile([1, 512], F32R, tag="mr")
                nc.vector.tensor_tensor(out=mr[:], in0=mu[:], in1=r[:],
                                        op=ALU.mult)
                bc = ps_bc.tile([128, 512], F32, space="PSUM", tag="bc")
                nc.tensor.matmul(bc[:], ones_row[:], r[:], start=True,
                                 stop=True)
                bc2 = ps_bc.tile([128, 512], F32, space="PSUM", tag="bc")
                nc.tensor.matmul(bc2[:], ones_row[:], mr[:], start=True,
                                 stop=True)
                for kt in range(CT):
                    t = gx2.tile([128, 512], F32, tag="lnt")
                    nc.vector.tensor_tensor(out=t[:],
                                            in0=xin[:, kt, qs].bitcast(F32),
                                            in1=bc[:], op=ALU.mult)
                    nc.vector.tensor_tensor(out=xout[:, kt, qs], in0=t[:],
                                            in1=bc2[:], op=ALU.subtract)

        def ln_pools():
            i = ln_ctr[0]
            ln_ctr[0] += 1
            ps_stat = tc.alloc_tile_pool(name=f"ps_st{i}", bufs=2,
                                         space="PSUM")
            ps_bc = tc.alloc_tile_pool(name=f"ps_bc{i}", bufs=2, space="PSUM")
            return ps_stat, ps_bc

        for l in range(n_layers):
            pa = tc.alloc_tile_pool(name=f"pa{l}", bufs=1)

            # ---- LN1 -> xn (bf16), interleaved with q,k projections ----
            xn = gxn.tile([128, CT, T], BF16, tag="xn")
            qkT = pa.tile([128, 12, T], BF16, tag="qkT")
            st1, bc1 = ln_pools()

            def qkv_half(qc):
                qs = slice(qc * 512, (qc + 1) * 512)
                ln_qc(xT, xn, eps_t[:], 1.0, qc, st1, bc1)
                for s in range(12):
                    w = gw.tile([128, CT, 128], BF16, tag="w6", name="w")
                    nc.sync.dma_start(out=w[:],
                                      in_=wq[l].ap()[s * 128:(s + 1) * 128])
                    ps = ps_big.tile([128, 512], F32, space="PSUM", tag="px",
                                     name="ps")
                    for kt in range(CT):
                        nc.tensor.matmul(ps[:], w[:, kt, :], xn[:, kt, qs],
                                         start=(kt == 0), stop=(kt == CT - 1))
                    nc.vector.tensor_copy(out=qkT[:, s, qs], in_=ps[:])

            # ---- v projection (tokens on PSUM partitions) ----
            wv = gwv.tile([128, CT, C], BF16, tag="vw")
            nc.sync.dma_start(out=wv[:], in_=wv_[l].ap()[:])

            def v_half(tts):
              for tt in tts:
                tsl = slice(tt * 128, (tt + 1) * 128)
                psA = ps_big.tile([128, 512], F32, space="PSUM", tag="px")
                for kt in range(CT):
                    nc.tensor.matmul(psA[:], xn[:, kt, tsl],
                                     wv[:, kt, 0:512],
                                     start=(kt == 0), stop=(kt == CT - 1))
                psB = ps_big.tile([128, 256], F32, space="PSUM", tag="px")
                for kt in range(CT):
                    nc.tensor.matmul(psB[:], xn[:, kt, tsl],
                                     wv[:, kt, 512:768],
                                     start=(kt == 0), stop=(kt == CT - 1))
                nc.vector.tensor_copy(
                    out=vT[:, tt, 0:8, 0:64],
                    in_=psA[:].rearrange("p (h d) -> p h d", h=8))
                nc.vector.tensor_copy(
                    out=vT[:, tt, 8:12, 0:64],
                    in_=psB[:].rearrange("p (h d) -> p h d", h=4))

            # ---- attention ----
            yT = pa.tile([128, CT, T], BF16, tag="yT")

            def attn_part(jlist):
              for hp in range(6):
                for j in jlist:
                    js = slice(j * AQ, (j + 1) * AQ)
                    epairs = {}
                    for p_ in range(j + 1):
                        pstiles = {}
                        for h in (2 * hp, 2 * hp + 1):
                            par = h % 2
                            rows = slice(64 * par, 64 * par + 64)
                            sps = ps_sc.tile([128, 2 * AQ], F32,
                                             space="PSUM", tag="sc",
                                             name="sps")
                            diag = (p_ == j)
                            for half in range(2):
                                kt = 2 * p_ + half
                                nc.tensor.matmul(
                                    sps[:, half * AQ:(half + 1) * AQ],
                                    qkT[rows, 6 + hp,
                                        kt * 128:(kt + 1) * 128],
                                    qkT[rows, hp, js],
                                    start=True, stop=not diag,
                                    skip_group_check=diag)
                                if diag:
                                    nc.tensor.matmul(
                                        sps[:, half * AQ:(half + 1) * AQ],
                                        iden_t[:],
                                        masks_t[:, half, :].rearrange(
                                            "p (i q) -> p i q", i=1)
                                        .broadcast_to([128, 2, AQ]),
                                        start=False, stop=True, perf_mode=DR,
                                        skip_group_check=True)
                            pstiles[h] = sps
                        for h in (2 * hp, 2 * hp + 1):
                            e = pE.tile([128, 2 * AQ], BF16, tag="E",
                                        name="e")
                            nc.scalar.activation(e[:], pstiles[h][:],
                                                 ACT.Exp, scale=0.125)
                            epairs[(h, p_)] = e
                    for h in (2 * hp, 2 * hp + 1):
                        par = h % 2
                        yrow = slice(64 * par, 64 * par + 64)
                        yps = ps_av.tile([128, AQ], F32, space="PSUM",
                                         tag="av", name="yps")
                        for kt in range(2 * j + 2):
                            e = epairs[(h, kt // 2)]
                            nc.tensor.matmul(
                                yps[:], vT[:, kt, h, :],
                                e[:, (kt % 2) * AQ:(kt % 2 + 1) * AQ],
                                start=(kt == 0), stop=(kt == 2 * j + 1))
                        rec = prec.tile([64, AQ], F32, tag="rec")
                        nc.vector.reciprocal(rec[:], yps[64:128, :])
                        nc.vector.tensor_tensor(out=yT[yrow, hp, js],
                                                in0=yps[0:64, :], in1=rec[:],
                                                op=ALU.mult)

            qkv_half(0)
            qkv_half(1)
            for p in (bc1, st1):
                p.release()
            pE = tc.alloc_tile_pool(name=f"pE{l}", bufs=8)
            prec = tc.alloc_tile_pool(name=f"prec{l}", bufs=3)
            ps_sc = tc.alloc_tile_pool(name=f"ps_sc{l}", bufs=3, space="PSUM")
            ps_av = tc.alloc_tile_pool(name=f"ps_av{l}", bufs=2, space="PSUM")
            v_half(range(0, 4))
            attn_part([0, 1])
            v_half(range(4, 8))
            attn_part([2, 3])
            for p in (ps_av, ps_sc, prec, pE):
                p.release()

            # ---- proj + residual, then LN2 + MLP, all qc-major ----
            xn2 = gxn.tile([128, CT, T], BF16, tag="xn")
            for qc in range(2):
                qs = slice(qc * 512, (qc + 1) * 512)
                for ot in range(CT):
                    w = gw.tile([128, CT, 128], BF16, tag="w6")
                    nc.sync.dma_start(out=w[:],
                                      in_=wp[l].ap()[ot * 128:(ot + 1) * 128])
                    ps = ps_big.tile([128, 512], F32, space="PSUM", tag="px")
                    for kt in range(CT):
                        nc.tensor.matmul(ps[:], w[:, kt, :], yT[:, kt, qs],
                                         start=(kt == 0), stop=(kt == CT - 1))
                    nc.vector.tensor_tensor(out=xT[:, ot, qs], in0=ps[:],
                                            in1=xT[:, ot, qs], op=ALU.add)
            pa.release()
            pm = tc.alloc_tile_pool(name=f"pm{l}", bufs=1)
            hT = pm.tile([128, HT, T], BF16, tag="hT")
            st2, bc2p = ln_pools()
            for qc in range(2):
                qs = slice(qc * 512, (qc + 1) * 512)
                ln_qc(xT, xn2, eps_t[:], 1.0, qc, st2, bc2p)
                for ot in range(HT):
                    w = gw.tile([128, CT, 128], BF16, tag="w6")
                    nc.sync.dma_start(out=w[:],
                                      in_=wf[l].ap()[ot * 128:(ot + 1) * 128])
                    ps = ps_big.tile([128, 512], F32, space="PSUM", tag="px")
                    for kt in range(CT):
                        nc.tensor.matmul(ps[:], w[:, kt, :], xn2[:, kt, qs],
                                         start=(kt == 0), stop=(kt == CT - 1))
                    nc.scalar.activation(hT[:, ot, qs], ps[:], ACT.Gelu)
            for p in (bc2p, st2):
                p.release()
            for qc in range(2):
                qs = slice(qc * 512, (qc + 1) * 512)
                for ot in range(CT):
                    w2 = gw24.tile([128, HT, 128], BF16, tag="w24")
                    nc.sync.dma_start(out=w2[:],
                                      in_=wf2[l].ap()[ot * 128:(ot + 1) * 128])
                    ps = ps_big.tile([128, 512], F32, space="PSUM", tag="px")
                    for kt in range(HT):
                        nc.tensor.matmul(ps[:], w2[:, kt, :], hT[:, kt, qs],
                                         start=(kt == 0), stop=(kt == HT - 1))
                    nc.vector.tensor_tensor(out=xT[:, ot, qs], in0=ps[:],
                                            in1=xT[:, ot, qs], op=ALU.add)
            pm.release()

        # ---- final LN (bf16, scale SXH) -> hi/lo fp8 split + LM head ----
        for p in (gv, gw24, gwv, gw, gxn):
            p.release()
        pf = tc.alloc_tile_pool(name="pf", bufs=1)
        xfl = pf.tile([128, CT, T], BF16, tag="xfl")
        stf, bcf = ln_pools()
        for qc in range(2):
            ln_qc(xT, xfl, epsh_t[:], SXH, qc, stf, bcf)
        for p in (bcf, stf):
            p.release()
        xf = pf.tile([128, CT, T], FP8, tag="xf")
        xl = pf.tile([128, CT, T], FP8, tag="xl")
        for kt in range(CT):
            nc.vector.tensor_copy(out=xf[:, kt, :], in_=xfl[:, kt, :])
        for kt in range(CT):
            nc.vector.tensor_tensor(out=xl[:, kt, :], in0=xfl[:, kt, :],
                                    in1=xf[:, kt, :], op=ALU.subtract)

        ph = tc.alloc_tile_pool(name="ph", bufs=4)
        pout = tc.alloc_tile_pool(name="pout", bufs=2)
        for vc in range(NVC):
            wh8 = ph.tile([128, KP, 2, 512], FP8, tag="wh")
            nc.sync.dma_start(out=wh8[:],
                              in_=whh_d.ap()[vc * 128:(vc + 1) * 128])
            wl8 = ph.tile([128, KP, 2, 512], FP8, tag="whl")
            nc.sync.dma_start(out=wl8[:],
                              in_=whl_d.ap()[vc * 128:(vc + 1) * 128])
            o = pout.tile([128, TT, 512], F32, tag="out")
            for tt in range(TT):
                tsl = slice(tt * 128, (tt + 1) * 128)
                ps = ps_big.tile([128, 512], F32, space="PSUM", tag="px")
                terms = [(xf, wh8), (xl, wh8), (xf, wl8)]
                for cc in range(2):
                    i = 0
                    for xsrc, wsrc in terms:
                        for kp in range(KP):
                            nc.tensor.matmul(
                                ps[:, cc * 256:(cc + 1) * 256],
                                xsrc[:, 2 * kp:2 * kp + 2, tsl],
                                wsrc[:, kp, :, cc * 256:cc * 256 + 256],
                                start=(i == 0), stop=(i == 3 * KP - 1),
                                perf_mode=DR)
                            i += 1
                if tt % 2 == 0:
                    nc.vector.tensor_scalar_mul(o[:, tt, :], ps[:],
                                                sc["dq_h"])
                else:
                    nc.scalar.mul(o[:, tt, :], ps[:], sc["dq_h"])
            nc.sync.dma_start(
                out=logits_d.ap()[:, vc * 512:(vc + 1) * 512].rearrange(
                    "(t p) v -> p t v", p=128),
                in_=o[:])
        for p in (pout, ph, pf, ps_big, gmicro, gx2, gx, glob):
            p.release()

    nc.compile()
    return nc


# ---------------------------------------------------------------------------
# host side
# ---------------------------------------------------------------------------

def _prep_inputs(inputs, n_layers):
    f32 = np.float32
    bf = ml_dtypes.bfloat16
    idx = np.asarray(inputs["idx"])
    wte = np.asarray(inputs["wte"], f32)
    wpe = np.asarray(inputs["wpe"], f32)

    sc = {}
    common = {}
    for l in range(n_layers):
        ln1w = np.asarray(inputs["ln1_w"][l], f32)
        ln1b = np.asarray(inputs["ln1_b"][l], f32)
        aw = np.asarray(inputs["attn_w"][l], f32)
        ab = np.asarray(inputs["attn_b"][l], f32)
        awf = ln1w[:, None] * aw
        abf = ab + ln1b @ aw
        assert not np.any(abf), "nonzero attn bias not supported"
        qk = awf[:, :1536]
        common[f"qkw{l}"] = np.ascontiguousarray(
            qk.reshape(CT, 128, 12, 128).transpose(2, 1, 0, 3)
        ).reshape(12 * 128, CT, 128).astype(bf)
        common[f"vw{l}"] = np.ascontiguousarray(
            awf[:, 1536:].reshape(CT, 128, C).transpose(1, 0, 2)).astype(bf)
        pw = np.asarray(inputs["proj_w"][l], f32)
        assert not np.any(np.asarray(inputs["proj_b"][l])), "proj bias"
        common[f"pw{l}"] = np.ascontiguousarray(
            pw.reshape(CT, 128, CT, 128).transpose(2, 1, 0, 3)
        ).reshape(CT * 128, CT, 128).astype(bf)
        ln2w = np.asarray(inputs["ln2_w"][l], f32)
        ln2b = np.asarray(inputs["ln2_b"][l], f32)
        fw = np.asarray(inputs["fc_w"][l], f32)
        fbv = np.asarray(inputs["fc_b"][l], f32)
        fwf = ln2w[:, None] * fw
        fbf = fbv + ln2b @ fw
        assert not np.any(fbf), "nonzero fc bias not supported"
        common[f"fw{l}"] = np.ascontiguousarray(
            fwf.reshape(CT, 128, HT, 128).transpose(2, 1, 0, 3)
        ).reshape(HT * 128, CT, 128).astype(bf)
        f2w = np.asarray(inputs["fc2_w"][l], f32)
        assert not np.any(np.asarray(inputs["fc2_b"][l])), "fc2 bias"
        common[f"f2w{l}"] = np.ascontiguousarray(
            f2w.reshape(HT, 128, CT, 128).transpose(2, 1, 0, 3)
        ).reshape(CT * 128, HT, 128).astype(bf)

    p = np.arange(128)[:, None]
    f = np.arange(AQ)[None, :]
    masks = np.zeros((128, 2, AQ), f32)
    masks[:, 0, :] = np.where(p > f, -240.0, 0.0)
    masks[:, 1, :] = np.where(p + 128 > f, -240.0, 0.0)
    common["masks"] = masks.astype(E4)
    iden = np.zeros((128, 2, 128), f32)
    iden[:, 0, :] = 128.0 * np.eye(128)
    common["iden"] = iden.astype(E4)

    lnfw = np.asarray(inputs["lnf_w"], f32)
    lnfb = np.asarray(inputs["lnf_b"], f32)
    assert not np.any(lnfb @ wte.T), "nonzero head bias not supported"
    wh = lnfw[:, None] * wte.T                     # [768, V]
    whp = np.zeros((C, 2 * VS), f32)
    whp[:, :V] = wh
    m = float(np.abs(wh).max())
    s_h = float(2.0 ** np.floor(np.log2(240.0 / m)))
    sc["dq_h"] = 1.0 / (s_h * SXH)

    whead, wheadl = {}, {}
    for vh in range(2):
        sl = whp[:, vh * VS:(vh + 1) * VS] * s_h
        hi = sl.astype(E4)
        lo = (sl - hi.astype(f32)).astype(E4)
        whead[vh] = np.ascontiguousarray(
            hi.reshape(KP, 2, 128, NVC, 512).transpose(3, 2, 0, 1, 4)
        ).reshape(NVC * 128, KP, 2, 512)
        wheadl[vh] = np.ascontiguousarray(
            lo.reshape(KP, 2, 128, NVC, 512).transpose(3, 2, 0, 1, 4)
        ).reshape(NVC * 128, KP, 2, 512)

    def t6(a):          # [768, T] -> [128, 6, T]
        return np.ascontiguousarray(
            a.reshape(CT, 128, a.shape[1]).transpose(1, 0, 2))

    x0 = wte[idx] + wpe[None, :T]                  # [B, T, C]
    in_maps = []
    for c in range(8):
        s, vh = c & 3, c >> 2
        m2 = dict(common)
        m2["x0t"] = t6(np.ascontiguousarray(x0[s].T))
        m2["whh"] = whead[vh]
        m2["whl"] = wheadl[vh]
        in_maps.append(m2)
    return in_maps, sc


def kernel(**inputs):
    global LAST_RESULT, LAST_NC
    n_layers = L
    in_maps, sc = _prep_inputs(inputs, n_layers)
    key = (n_layers, tuple(sorted(sc.items())))
    if key not in _CACHE:
        _CACHE[key] = build_program(sc, n_layers)
    nc = _CACHE[key]
    LAST_NC = nc
    res = run_bass_kernel_spmd(nc, in_maps, core_ids=list(range(8)))
    LAST_RESULT = res
    out = np.empty((B, T, V), np.float32)
    for c in range(8):
        s, vh = c & 3, c >> 2
        part = res.results[c]["logits"]
        if vh == 0:
            out[s, :, :VS] = part
        else:
            out[s, :, VS:] = part[:, :V - VS]
    return out


if __name__ == "__main__":
    rng = np.random.default_rng(0)
    ins = {
        "idx": rng.integers(0, V, (B, T)).astype(np.int32),
        "wte": (rng.standard_normal((V, C)) * 0.02).astype(np.float32),
        "wpe": (rng.standard_normal((T, C)) * 0.02).astype(np.float32),
        "ln1_w": np.ones((L, C), np.float32),
        "ln1_b": np.zeros((L, C), np.float32),
        "attn_w": (rng.standard_normal((L, C, 3 * C)) * 0.02).astype(np.float32),
        "attn_b": np.zeros((L, 3 * C), np.float32),
        "proj_w": (rng.standard_normal((L, C, C)) * 0.02).astype(np.float32),
        "proj_b": np.zeros((L, C), np.float32),
        "ln2_w": np.ones((L, C), np.float32),
        "ln2_b": np.zeros((L, C), np.float32),
        "fc_w": (rng.standard_normal((L, C, 4 * C)) * 0.02).astype(np.float32),
        "fc_b": np.zeros((L, 4 * C), np.float32),
        "fc2_w": (rng.standard_normal((L, 4 * C, C)) * 0.02).astype(np.float32),
        "fc2_b": np.zeros((L, C), np.float32),
        "lnf_w": np.ones((C,), np.float32),
        "lnf_b": np.zeros((C,), np.float32),
    }
    out = kernel(**ins)
    print("out", out.shape, out.dtype, float(np.abs(out).max()))

